# revision 22
# baseline (speedup 1.0000x reference)
"""Trainium2 Bass kernel for AttentionReadout2DPDE.

Reference computation (per sample b):
    hid  = relu(measurement @ W1 + b1)                       [B, H]
    raw  = (hid @ W2 + b2).reshape(B, Q, 2 + D)
    xy   = sigmoid(raw[:, :, :2])                            [B, Q, 2]
    w    = raw[:, :, 2:]                                     [B, Q, D]
    mu, sd = mean/std(field_u[b])  (std unbiased, clamp 1e-6)
    pde  = bilinear_sample((field_u - mu) / sd, xy)          [B, Q]
    out  = einsum('bq,bqd->bd', pde, w)                      [B, D]

Key fact used: bilinear weights sum to 1, so
    bilinear(field_norm) = (bilinear(field_u) - mu) / sd
and the normalized field never needs to be materialized.  The kernel
runs the MLP -> query offsets -> one batched indirect gather (4 corner
values per query, 8-byte descriptors) first, then streams each sample's
field once for the statistics (sum on VectorE, sum-of-squares on
ScalarE via activation accum), alternating the field DMAs between the
two HWDGE queues (sync / scalar) so the stream stays at HBM rate.

W2/b2 are pre-split on the host into the xy columns (query positions)
and the D weight columns, so the position matmuls need no strided
channel views and the gather issue depends only on a small early chain.

Sharding: pure data parallel, batch 256 -> 8 cores x 32 samples.
"""

import numpy as np
from contextlib import ExitStack

import concourse.bass as bass
import concourse.tile as tile
import concourse.mybir as mybir
from concourse import bacc
from concourse.bass_utils import run_bass_kernel_spmd
from concourse.masks import make_identity

F32 = mybir.dt.float32
I32 = mybir.dt.int32
AF = mybir.ActivationFunctionType
OP = mybir.AluOpType
AX = mybir.AxisListType

B, S, NX, NY = 256, 256, 512, 512
Q, D, H = 64, 32, 256
CH = 2 + D            # 34 channels per query
NCORES = 8
BL = B // NCORES      # 32 samples per core
FS = NX * NY          # 262144 field elems per sample
P = 128
COLS = FS // P        # 2048 field elems per partition per sample
SPD = 4               # samples per field DMA (4 MB transfers)
FBUFS = 3             # field tile ring depth
NG = BL // 2          # gather DMAs (2 samples per DMA)
GW = 520              # padded gather run length (514 used)

PARTS = {"mlp", "gath", "field", "stats", "combine"}


def _body(ctx: ExitStack, tc: "tile.TileContext", meas_d, field_d, w1_d, b1_d,
          w2xy_d, b2xy_d, w2d_d, b2d_d, bbase_d, sel_d, out_d, repeat=1):
    nc = tc.nc
    const = ctx.enter_context(tc.tile_pool(name="const", bufs=1))
    spool = ctx.enter_context(tc.tile_pool(name="small", bufs=1))
    fpool = ctx.enter_context(tc.tile_pool(name="field", bufs=FBUFS))
    scr = ctx.enter_context(tc.tile_pool(name="scratch", bufs=1))
    psum = ctx.enter_context(tc.tile_pool(name="psum", bufs=3, space="PSUM"))

    # ---------------- constants / weights ----------------
    # Small tensors go on the SWDGE (gpsimd) queue; the big w2d matrix on
    # the scalar HWDGE queue so neither blocks the early MLP chain.
    w1_sb = const.tile([P, 2, H], F32)
    w2xy_sb = const.tile([P, 2, 4 * Q], F32)
    w2d_sb = const.tile([P, 2, Q * D], F32)
    b1_sb = const.tile([P, 2], F32)
    b2xy_sb = const.tile([1, 4 * Q], F32)
    b2d_sb = const.tile([1, Q * D], F32)
    meas_sb = const.tile([BL, S], F32)
    bbase_sb = const.tile([P, NG], F32)
    sel_sb = const.tile([NG, 2 * BL], F32)
    ident = const.tile([P, P], F32)
    ones1 = const.tile([1, Q], F32)
    nc.gpsimd.dma_start(out=meas_sb[:], in_=meas_d[:])
    for k in range(2):
        nc.gpsimd.dma_start(out=w1_sb[:, k, :], in_=w1_d[k * P:(k + 1) * P, :])
        nc.gpsimd.dma_start(out=w2xy_sb[:, k, :],
                            in_=w2xy_d[k * P:(k + 1) * P, :])
        nc.gpsimd.dma_start(out=b1_sb[:, k:k + 1],
                            in_=b1_d[k * P:(k + 1) * P, None])
        nc.scalar.dma_start(out=w2d_sb[:, k, :],
                            in_=w2d_d[k * P:(k + 1) * P, :])
    nc.gpsimd.dma_start(out=b2xy_sb[:], in_=b2xy_d[None, :])
    nc.gpsimd.dma_start(out=b2d_sb[:], in_=b2d_d[None, :])
    nc.gpsimd.dma_start(out=bbase_sb[:], in_=bbase_d[:])
    nc.gpsimd.dma_start(out=sel_sb[:], in_=sel_d[:])
    make_identity(nc, ident[:])
    nc.gpsimd.memset(ones1[:], 1.0)

    field_flat = field_d[:].rearrange("b y x -> (b y x)")[None, :]

    def _compute():
        exy_ps = None
        Asum = Csum = None
        if "mlp" in PARTS:
            # ---------------- MLP ----------------
            # measT[s, b] via PE transpose (two 32x128 -> 128x32 chunks)
            measT_sb = spool.tile([P, 2, BL], F32, tag="measT")
            for k in range(2):
                mt_ps = psum.tile([P, BL], F32, tag="mm")
                nc.tensor.transpose(out=mt_ps[:],
                                    in_=meas_sb[:, k * P:(k + 1) * P],
                                    identity=ident[0:BL, 0:BL])
                nc.vector.tensor_copy(out=measT_sb[:, k, :], in_=mt_ps[:])

            # hidT[h, b] = relu(W1.T @ measT + b1)
            hidT_sb = spool.tile([P, 2, BL], F32, tag="hidT")
            for hk in range(2):
                h_ps = psum.tile([P, BL], F32, tag="mm")
                for sk in range(2):
                    nc.tensor.matmul(out=h_ps[:],
                                     lhsT=w1_sb[:, sk, hk * P:(hk + 1) * P],
                                     rhs=measT_sb[:, sk, :],
                                     start=(sk == 0), stop=(sk == 1))
                nc.scalar.activation(out=hidT_sb[:, hk, :], in_=h_ps[:],
                                     func=AF.Relu, bias=b1_sb[:, hk:hk + 1],
                                     scale=1.0)

            # ------------- query positions, (row, q)-on-partition layout ----
            # Partition p = t*64 + q holds query q's position for the top
            # (t=0) / bottom (t=1) bilinear row; the host duplicated the xy
            # weight columns so both halves come out of one matmul chain.
            pxt = {}
            for ci, name in ((0, "x"), (1, "y")):
                ps = psum.tile([P, BL], F32, tag="mm")
                for hk in range(2):
                    nc.tensor.matmul(out=ps[:],
                                     lhsT=w2xy_sb[:, hk, ci * P:(ci + 1) * P],
                                     rhs=hidT_sb[:, hk, :],
                                     start=(hk == 0), stop=False)
                nc.tensor.matmul(out=ps[:], lhsT=b2xy_sb[:, ci * P:(ci + 1) * P],
                                 rhs=ones1[:, 0:BL], start=False, stop=True)
                sg = spool.tile([P, BL], F32, tag=f"sig{name}")
                nc.scalar.activation(out=sg[:], in_=ps[:], func=AF.Sigmoid)
                p = spool.tile([P, BL], F32, tag=f"p{name}")
                nc.vector.tensor_scalar_mul(out=p[:], in0=sg[:],
                                            scalar1=float(NY - 1))
                pxt[name] = p

            # floor via the 2^23 magic-number round + is_gt fixup (exact for
            # 0 <= p < 2^22):  rnd = round_nearest(p); v0 = rnd - (rnd > p);
            # clamp to [0, 510]
            MAGIC = 8388608.0
            pos0 = {}
            wgt = {}
            for name in ("x", "y"):
                p = pxt[name]
                rnd1 = spool.tile([P, BL], F32, tag=f"rnd1{name}")
                nc.vector.tensor_scalar_add(out=rnd1[:], in0=p[:], scalar1=MAGIC)
                rnd = spool.tile([P, BL], F32, tag=f"rnd{name}")
                nc.vector.tensor_scalar_sub(out=rnd[:], in0=rnd1[:],
                                            scalar1=MAGIC)
                gm = spool.tile([P, BL], F32, tag=f"gm{name}")
                nc.vector.tensor_tensor(out=gm[:], in0=rnd[:], in1=p[:],
                                        op=OP.is_gt)
                v0 = spool.tile([P, BL], F32, tag=f"v0{name}")
                nc.vector.tensor_sub(out=v0[:], in0=rnd[:], in1=gm[:])
                v0c = spool.tile([P, BL], F32, tag=f"v0c{name}")
                nc.vector.tensor_scalar(out=v0c[:], in0=v0[:],
                                        scalar1=float(NY - 2), scalar2=0.0,
                                        op0=OP.min, op1=OP.max)
                w = spool.tile([P, BL], F32, tag=f"w{name}")
                nc.vector.tensor_sub(out=w[:], in0=p[:], in1=v0c[:])
                pos0[name] = v0c
                wgt[name] = w

            # off[p, b] = y0*512 + x0 (top-left corner, sans sample base)
            offa = spool.tile([P, BL], F32, tag="offa")
            nc.vector.tensor_scalar_mul(out=offa[:], in0=pos0["y"][:],
                                        scalar1=float(NY))
            offb = spool.tile([P, BL], F32, tag="offb")
            nc.vector.tensor_add(out=offb[:], in0=offa[:], in1=pos0["x"][:])

            # Gathers pack TWO samples per DMA: partition p = s*64 + q holds
            # sample b = 2j + s for DMA j.  De-interleave the per-(q, b)
            # offsets into that layout (even columns for the top half, odd
            # for the bottom) and add bbase[p, j] = (2j + s)*FS.
            def deint(src):
                v = src.rearrange("p (j s) -> p j s", s=2)
                return (v[:, :, 0:1].rearrange("p j s -> p (j s)"),
                        v[:, :, 1:2].rearrange("p j s -> p (j s)"))

            offsel = spool.tile([P, NG], F32, tag="offsel")
            ev, od = deint(offb[0:Q, :])
            nc.vector.tensor_add(out=offsel[0:Q, :], in0=ev,
                                 in1=bbase_sb[0:Q, :])
            ev, od = deint(offb[Q:P, :])
            nc.vector.tensor_add(out=offsel[Q:P, :], in0=od,
                                 in1=bbase_sb[Q:P, :])
            offi = spool.tile([P, NG], I32, tag="offi")
            nc.vector.tensor_copy(out=offi[:], in_=offsel[:])

            # weights in the same (s, q) x j layout
            wx16 = spool.tile([P, NG], F32, tag="wx16")
            ev, od = deint(wgt["x"][0:Q, :])
            nc.vector.tensor_copy(out=wx16[0:Q, :], in_=ev)
            ev, od = deint(wgt["x"][Q:P, :])
            nc.vector.tensor_copy(out=wx16[Q:P, :], in_=od)
            wy16 = spool.tile([P, NG], F32, tag="wy16")
            ev, od = deint(wgt["y"][0:Q, :])
            nc.vector.tensor_copy(out=wy16[0:Q, :], in_=ev)
            ev, od = deint(wgt["y"][Q:P, :])
            nc.vector.tensor_copy(out=wy16[Q:P, :], in_=od)

            # ---------------- paired-sample gathers ----------------
            # One indirect DMA per sample pair: 128 partitions (sample
            # parity, query), each fetching a contiguous 514-float run that
            # covers all 4 bilinear corners (cols 0, 1, 512, 513).  The
            # SWDGE ucode supports only one offset per partition per DMA
            # (multi-column offset APs gather garbage), so this cannot be
            # batched further.
            G5 = spool.tile([P, NG, GW], F32, tag="G5")
            if "gath" in PARTS:
                for j in range(NG):
                    nc.gpsimd.indirect_dma_start(
                        out=G5[:, j, 0:NY + 2], out_offset=None,
                        in_=field_flat,
                        in_offset=bass.IndirectOffsetOnAxis(
                            ap=offi[:, j:j + 1], axis=1))
            else:
                nc.gpsimd.memset(G5[:], 0.0)

            # ---------------- weight matrix + Csum ----------------
            rawd_sb = spool.tile([BL, Q * D], F32, tag="rawd")
            for off in range(0, Q * D, 512):
                r_ps = psum.tile([BL, 512], F32, tag="mm")
                for hk in range(2):
                    nc.tensor.matmul(out=r_ps[:], lhsT=hidT_sb[:, hk, :],
                                     rhs=w2d_sb[:, hk, off:off + 512],
                                     start=(hk == 0), stop=False)
                nc.tensor.matmul(out=r_ps[:], lhsT=ones1[:, 0:BL],
                                 rhs=b2d_sb[:, off:off + 512],
                                 start=False, stop=True)
                nc.vector.tensor_copy(out=rawd_sb[:, off:off + 512],
                                      in_=r_ps[:])
            Csum = spool.tile([BL, D], F32, tag="Csum")
            nc.vector.reduce_sum(
                out=Csum[:],
                in_=rawd_sb[:].rearrange("p (q d) -> p d q", d=D), axis=AX.X)

            # ---------------- bilinear combine ----------------
            # Full bilinear in the (s, q) x j layout (both corner rows live
            # in the same partition's run), then a PE transpose + two 0/1
            # permutation matmuls restore sample-on-partition order.
            def gcol(c):
                return G5[:, :, c:c + 1].rearrange("p j o -> p (j o)")

            d0 = spool.tile([P, NG], F32, tag="d0")
            nc.vector.tensor_sub(out=d0[:], in0=gcol(1), in1=gcol(0))
            m0 = spool.tile([P, NG], F32, tag="m0")
            nc.vector.tensor_mul(out=m0[:], in0=d0[:], in1=wx16[:])
            ex0 = spool.tile([P, NG], F32, tag="ex0")
            nc.vector.tensor_add(out=ex0[:], in0=gcol(0), in1=m0[:])
            d1 = spool.tile([P, NG], F32, tag="d1")
            nc.vector.tensor_sub(out=d1[:], in0=gcol(NY + 1), in1=gcol(NY))
            m1 = spool.tile([P, NG], F32, tag="m1")
            nc.vector.tensor_mul(out=m1[:], in0=d1[:], in1=wx16[:])
            ex1 = spool.tile([P, NG], F32, tag="ex1")
            nc.vector.tensor_add(out=ex1[:], in0=gcol(NY), in1=m1[:])
            dy = spool.tile([P, NG], F32, tag="dy")
            nc.vector.tensor_sub(out=dy[:], in0=ex1[:], in1=ex0[:])
            my = spool.tile([P, NG], F32, tag="my")
            nc.vector.tensor_mul(out=my[:], in0=dy[:], in1=wy16[:])
            exy16 = spool.tile([P, NG], F32, tag="exy16")
            nc.vector.tensor_add(out=exy16[:], in0=ex0[:], in1=my[:])

            exyT_ps = psum.tile([NG, P], F32, tag="tr")
            nc.tensor.transpose(out=exyT_ps[:], in_=exy16[:],
                                identity=ident[:])
            exyT = spool.tile([NG, P], F32, tag="exyT")
            nc.vector.tensor_copy(out=exyT[:], in_=exyT_ps[:])
            exy_ps = psum.tile([BL, Q], F32, tag="tr")
            nc.tensor.matmul(out=exy_ps[:], lhsT=sel_sb[:, 0:BL],
                             rhs=exyT[:, 0:Q], start=True, stop=False)
            nc.tensor.matmul(out=exy_ps[:], lhsT=sel_sb[:, BL:2 * BL],
                             rhs=exyT[:, Q:P], start=False, stop=True)

            # einsum('bq,bqd->bd') split so both reductions run early:
            #   out = inv * A + (-mu*inv) * C,
            #   A[b,d] = sum_q exy[b,q]*W[b,q,d],  C[b,d] = sum_q W[b,q,d]
            prodA = spool.tile([BL, Q * D], F32, tag="prodA")
            nc.vector.tensor_tensor(
                out=prodA[:].rearrange("p (q d) -> p q d", d=D),
                in0=exy_ps[:].rearrange("p (q o) -> p q o", o=1).to_broadcast(
                    [BL, Q, D]),
                in1=rawd_sb[:].rearrange("p (q d) -> p q d", d=D), op=OP.mult)
            Asum = spool.tile([BL, D], F32, tag="Asum")
            nc.vector.reduce_sum(
                out=Asum[:],
                in_=prodA[:].rearrange("p (q d) -> p d q", d=D), axis=AX.X)

        # ---------------- field statistics (the memory-bound stream) ------
        part_s = spool.tile([P, BL], F32, tag="part_s")
        part_q = spool.tile([P, BL], F32, tag="part_q")
        if "field" in PARTS:
            for t in range(BL // SPD):
                ft = fpool.tile([P, SPD * COLS], F32, tag="ft")
                eng = nc.sync if (t % 2 == 0) else nc.scalar
                eng.dma_start(
                    out=ft[:].rearrange("p (b ay) -> p b ay", b=SPD),
                    in_=field_d[t * SPD:(t + 1) * SPD].rearrange(
                        "b (p a) y -> p b (a y)", p=P))
                if "stats" not in PARTS:
                    continue
                for s in range(SPD):
                    b = t * SPD + s
                    nc.vector.reduce_sum(out=part_s[:, b:b + 1],
                                         in_=ft[:, s * COLS:(s + 1) * COLS],
                                         axis=AX.X)
                    sq = scr.tile([P, COLS], F32, tag="sq")
                    nc.scalar.activation(out=sq[:],
                                         in_=ft[:, s * COLS:(s + 1) * COLS],
                                         func=AF.Square,
                                         accum_out=part_q[:, b:b + 1])

        if "combine" in PARTS and "mlp" in PARTS:
            # cross-partition aggregation: PE transpose + free-dim reduce
            ts_ps = psum.tile([BL, P], F32, tag="tr")
            nc.tensor.transpose(out=ts_ps[:], in_=part_s[:], identity=ident[:])
            tq_ps = psum.tile([BL, P], F32, tag="tr")
            nc.tensor.transpose(out=tq_ps[:], in_=part_q[:], identity=ident[:])
            Ssum = spool.tile([BL, 1], F32, tag="Ssum")
            Qsum = spool.tile([BL, 1], F32, tag="Qsum")
            nc.vector.reduce_sum(out=Ssum[:], in_=ts_ps[:], axis=AX.X)
            nc.vector.reduce_sum(out=Qsum[:], in_=tq_ps[:], axis=AX.X)

            # mu = S/N ; var = (Q - S^2/N)/(N-1) ; sd = max(sqrt(var), 1e-6)
            mu = spool.tile([BL, 1], F32, tag="mu")
            nc.vector.tensor_scalar_mul(out=mu[:], in0=Ssum[:],
                                        scalar1=1.0 / FS)
            s2 = spool.tile([BL, 1], F32, tag="s2")
            nc.vector.tensor_mul(out=s2[:], in0=Ssum[:], in1=mu[:])
            varn = spool.tile([BL, 1], F32, tag="varn")
            nc.vector.tensor_sub(out=varn[:], in0=Qsum[:], in1=s2[:])
            var = spool.tile([BL, 1], F32, tag="var")
            nc.vector.tensor_scalar_mul(out=var[:], in0=varn[:],
                                        scalar1=1.0 / (FS - 1))
            sd = spool.tile([BL, 1], F32, tag="sd")
            nc.scalar.activation(out=sd[:], in_=var[:], func=AF.Sqrt)
            sdc = spool.tile([BL, 1], F32, tag="sdc")
            nc.vector.tensor_scalar_max(out=sdc[:], in0=sd[:], scalar1=1e-6)
            inv = spool.tile([BL, 1], F32, tag="inv")
            nc.vector.reciprocal(out=inv[:], in_=sdc[:])
            nmi0 = spool.tile([BL, 1], F32, tag="nmi0")
            nc.vector.tensor_mul(out=nmi0[:], in0=mu[:], in1=inv[:])
            nmi = spool.tile([BL, 1], F32, tag="nmi")
            nc.vector.tensor_scalar_mul(out=nmi[:], in0=nmi0[:], scalar1=-1.0)

            # out = inv*A + nmi*C  (tiny tail; A and C were reduced early)
            tA = spool.tile([BL, D], F32, tag="tA")
            nc.vector.tensor_scalar(out=tA[:], in0=Asum[:],
                                    scalar1=inv[:, 0:1], scalar2=None,
                                    op0=OP.mult)
            tC = spool.tile([BL, D], F32, tag="tC")
            nc.vector.tensor_scalar(out=tC[:], in0=Csum[:],
                                    scalar1=nmi[:, 0:1], scalar2=None,
                                    op0=OP.mult)
            outt = spool.tile([BL, D], F32, tag="outt")
            nc.vector.tensor_add(out=outt[:], in0=tA[:], in1=tC[:])
            nc.sync.dma_start(out=out_d[:], in_=outt[:])

    for _ in range(repeat):
        _compute()


def build(repeat: int = 1):
    nc = bacc.Bacc("TRN2", target_bir_lowering=False, debug=False,
                   num_devices=NCORES)
    meas_d = nc.dram_tensor("meas", [BL, S], F32, kind="ExternalInput").ap()
    field_d = nc.dram_tensor("field", [BL, NX, NY], F32,
                             kind="ExternalInput").ap()
    w1_d = nc.dram_tensor("w1", [S, H], F32, kind="ExternalInput").ap()
    b1_d = nc.dram_tensor("b1", [H], F32, kind="ExternalInput").ap()
    w2xy_d = nc.dram_tensor("w2xy", [H, 4 * Q], F32,
                            kind="ExternalInput").ap()
    b2xy_d = nc.dram_tensor("b2xy", [4 * Q], F32, kind="ExternalInput").ap()
    w2d_d = nc.dram_tensor("w2d", [H, Q * D], F32, kind="ExternalInput").ap()
    b2d_d = nc.dram_tensor("b2d", [Q * D], F32, kind="ExternalInput").ap()
    bbase_d = nc.dram_tensor("bbase", [P, NG], F32, kind="ExternalInput").ap()
    sel_d = nc.dram_tensor("sel", [NG, 2 * BL], F32,
                           kind="ExternalInput").ap()
    out_d = nc.dram_tensor("out", [BL, D], F32, kind="ExternalOutput").ap()
    with tile.TileContext(nc) as tc:
        with ExitStack() as ctx:
            _body(ctx, tc, meas_d, field_d, w1_d, b1_d, w2xy_d, b2xy_d,
                  w2d_d, b2d_d, bbase_d, sel_d, out_d, repeat=repeat)
    nc.compile()
    return nc


_CACHE = {}


def _get_nc():
    if "nc" not in _CACHE:
        _CACHE["nc"] = build()
    return _CACHE["nc"]


def make_in_maps(measurement, field_u, W1, b1, W2, b2):
    ms = np.ascontiguousarray(np.asarray(measurement, np.float32))
    fu = np.ascontiguousarray(np.asarray(field_u, np.float32))
    w1 = np.ascontiguousarray(np.asarray(W1, np.float32))
    b1a = np.ascontiguousarray(np.asarray(b1, np.float32))
    w2 = np.asarray(W2, np.float32).reshape(H, Q, CH)
    b2a = np.asarray(b2, np.float32).reshape(Q, CH)
    # split W2/b2 into the xy columns and the D block; duplicate the x and
    # y column blocks so one matmul fills both (row, q) partition halves:
    # layout [x, x, y, y], each block Q wide
    w2x = w2[:, :, 0]
    w2y = w2[:, :, 1]
    w2xy = np.ascontiguousarray(
        np.concatenate([w2x, w2x, w2y, w2y], axis=1))
    w2d = np.ascontiguousarray(w2[:, :, 2:].reshape(H, Q * D))
    b2x = b2a[:, 0]
    b2y = b2a[:, 1]
    b2xy = np.ascontiguousarray(np.concatenate([b2x, b2x, b2y, b2y]))
    b2d = np.ascontiguousarray(b2a[:, 2:].reshape(Q * D))
    # bbase[p, j] = (2j + s)*FS with s = p // 64 (sample parity half)
    bbase = (np.arange(NG, dtype=np.float32) * 2 * FS)[None, :] \
        + (np.arange(P, dtype=np.float32)[:, None] // Q) * FS
    bbase = np.ascontiguousarray(bbase.astype(np.float32))
    # 0/1 permutation matrices: sel[:, 0:BL][j, 2j] = 1, sel[:, BL:][j, 2j+1] = 1
    sel = np.zeros((NG, 2 * BL), np.float32)
    sel[np.arange(NG), 2 * np.arange(NG)] = 1.0
    sel[np.arange(NG), BL + 2 * np.arange(NG) + 1] = 1.0
    in_maps = []
    for c in range(NCORES):
        sl = slice(c * BL, (c + 1) * BL)
        in_maps.append({
            "meas": np.ascontiguousarray(ms[sl]),
            "field": np.ascontiguousarray(fu[sl]),
            "w1": w1, "b1": b1a, "w2xy": w2xy, "b2xy": b2xy,
            "w2d": w2d, "b2d": b2d, "bbase": bbase, "sel": sel,
        })
    return in_maps


def kernel(measurement, field_u, W1, b1, W2, b2):
    nc = _get_nc()
    in_maps = make_in_maps(measurement, field_u, W1, b1, W2, b2)
    res = run_bass_kernel_spmd(nc, in_maps, core_ids=list(range(NCORES)))
    return np.concatenate([r["out"] for r in res.results], axis=0)
